# revision 37
# baseline (speedup 1.0000x reference)
"""BERT self-attention Bass kernel for 8 Trainium2 NeuronCores.

Problem: B=4, S=2048, D=1024, H=16, HD=64 fp32.
Sharding: core c -> batch b=c//2, head-half hh=c%2 (heads hh*8..hh*8+8).

Per-core dataflow (everything fp32):
  - host pre-transposes hidden[b] -> ht [D=1024, T=2048] (d-major)
  - Q^T, K^T projections: out [j(part), t(free)] with lhsT = W chunks
  - V projection: out [t(part), j(free)] with lhsT = ht chunks; V stored
    per (t-chunk, head) as [128, 64] with a 65th column of ones so the
    PV matmul's 65th output row accumulates the softmax denominator.
  - scores computed transposed: S_t[k(part), q(free)] = K_d^T-chunk.T @ Q_d
    (contract over hd=64); exp on ScalarE directly from PSUM with
    scale=1/8 and per-partition bias = attention-mask slice.
  - PV: ctx^T[hd+1, q] += V_aug-chunk.T @ E_t-chunk, accumulated over the
    16 k-chunks in PSUM.
  - out[h] = [65, 2048] (unnormalized ctx^T plus denominator row).
Host: ctx = out[:64]/out[64] + bv (exact: sum of probs is 1), transpose,
interleave heads into [B, S, D].
"""

import os
import sys

import numpy as np

for p in ("/opt/trn_rl_repo", "/root/.axon_site", "/root/.axon_site/_ro/trn_rl_repo"):
    if os.path.isdir(p) and p not in sys.path:
        sys.path.append(p)

import concourse.bacc as bacc
import concourse.bass as bass
import concourse.mybir as mybir
import concourse.tile as tile
from concourse.bass_utils import run_bass_kernel_spmd

B, S, D, H = 4, 2048, 1024, 16
HD = D // H  # 64
N_CORES = 8
P = 128
DJ = 512  # per-core head columns (8 heads * 64)
NH = 8  # heads per core
DC = D // P  # 8 d-in chunks
JC = DJ // P  # 4 j chunks (2 heads each)
TB = 4  # t blocks of 512 in projection
KC = S // P  # 16 k chunks
QW = 1024  # q tile width in attention
QC = S // QW  # 2
F32 = mybir.dt.float32
F32R = mybir.dt.float32r  # reduced-precision PE input: 1 cycle/row vs 4

_CACHE = {}


def build_nc():
    """Build + compile the SPMD single-core program (same for all cores)."""
    nc = bacc.Bacc("TRN2", target_bir_lowering=False, debug=False)

    # f32r inputs: same fp32 bits from the host; PE rounds on read and the
    # BIR verifier accepts DMA-from-f32r-DRAM as a rounded producer.
    ht_d = nc.declare_dram_parameter("ht", [D, S], F32R, isOutput=False)
    wq_d = nc.declare_dram_parameter("wq", [D, DJ], F32R, isOutput=False)
    wk_d = nc.declare_dram_parameter("wk", [D, DJ], F32R, isOutput=False)
    wv_d = nc.declare_dram_parameter("wv", [D, DJ], F32R, isOutput=False)
    bq_d = nc.declare_dram_parameter("bq", [DJ], F32, isOutput=False)
    bk_d = nc.declare_dram_parameter("bk", [DJ], F32, isOutput=False)
    mask_d = nc.declare_dram_parameter("mask", [S], F32, isOutput=False)
    out_d = nc.declare_dram_parameter("out", [NH, HD + 1, S], F32, isOutput=True)

    scr_d = nc.dram_tensor("scr", [32, 384], F32)  # keeps dummy matmuls live

    EXP = mybir.ActivationFunctionType.Exp

    with tile.TileContext(nc) as tc:
        with (
            tc.tile_pool(name="const", bufs=1) as const_pool,
            tc.tile_pool(name="w", bufs=1) as w_pool,
            tc.tile_pool(name="qk", bufs=1) as qk_pool,
            tc.tile_pool(name="v", bufs=1) as v_pool,
            tc.tile_pool(name="ht", bufs=2) as ht_pool,
            tc.tile_pool(name="e", bufs=4) as e_pool,
            tc.tile_pool(name="o", bufs=2) as o_pool,
            tc.tile_pool(name="ps", bufs=1, space="PSUM") as ps,
        ):
            # ---- constants ----
            bq_sb = const_pool.tile([P, JC], F32)
            nc.sync.dma_start(bq_sb, bq_d.ap().rearrange("(jc p) -> p jc", p=P))
            bk_sb = const_pool.tile([P, JC], F32)
            nc.sync.dma_start(bk_sb, bk_d.ap().rearrange("(jc p) -> p jc", p=P))
            mask_sb = const_pool.tile([P, KC], F32)
            nc.sync.dma_start(mask_sb, mask_d.ap().rearrange("(kc p) -> p kc", p=P))

            # ---- weights: V full; Q/K per j-chunk (lazily, 2 slots) ----
            wq_r = wq_d.ap().rearrange("(dc p) j -> p dc j", p=P)
            wk_r = wk_d.ap().rearrange("(dc p) j -> p dc j", p=P)
            wq_t, wk_t = {}, {}

            def load_wjc(jc):
                tk = w_pool.tile([P, DC, P], F32R, tag="wkjc", bufs=2, name=f"wk{jc}")
                nc.sync.dma_start(tk, wk_r[:, :, jc * P : (jc + 1) * P])
                wk_t[jc] = tk
                tq = w_pool.tile([P, DC, P], F32R, tag="wqjc", bufs=2, name=f"wq{jc}")
                nc.sync.dma_start(tq, wq_r[:, :, jc * P : (jc + 1) * P])
                wq_t[jc] = tq

            ht_r = ht_d.ap().rearrange("(dc p) t -> p dc t", p=P)

            def load_ht(tb):
                t = ht_pool.tile([P, DC, 512], F32R, tag="ht", name="ht_t")
                ts_ = slice(tb * 512, (tb + 1) * 512)
                nc.sync.dma_start(t[:, 0:4, :], ht_r[:, 0:4, ts_])
                nc.sync.dma_start(t[:, 4:DC, :], ht_r[:, 4:DC, ts_])
                return t

            # DMA order matters: the first projection matmul needs wk0 + ht
            # tile 0, so issue those before the bulk weight loads (HWDGE is
            # FIFO per issuing engine).
            load_wjc(0)
            ht_first = load_ht(0)
            wv_sb = w_pool.tile([P, DC, DJ], F32R)
            wv_r = wv_d.ap().rearrange("(dc p) j -> p dc j", p=P)
            nc.sync.dma_start(wv_sb[:, 0:4, :], wv_r[:, 0:4, :])
            nc.sync.dma_start(wv_sb[:, 4:DC, :], wv_r[:, 4:DC, :])
            load_wjc(1)

            # ---- persistent activations ----
            # Q^T / K^T per j-chunk: [j in chunk (part), t (free)]
            qd = [qk_pool.tile([P, S], F32R, name=f"qd{jc}") for jc in range(JC)]
            kd = [qk_pool.tile([P, S], F32R, name=f"kd{jc}") for jc in range(JC)]
            # V + ones column: [t within chunk (part), kchunk, head, hd+1]
            v_all = v_pool.tile([P, KC, NH, HD + 1], F32R)
            ones_sb = const_pool.tile([P, KC * NH], F32)
            nc.vector.memset(ones_sb, 1.0)
            nc.vector.tensor_copy(
                v_all[:, :, :, HD],
                ones_sb.rearrange("p (a b) -> p a b", b=NH),
            )

            def emit_qk_proj(jc, tb, ht_t, which, step=False):
                """Project one [128j, 512t] tile of Q^T or K^T. When used as
                a generator (step=True) it yields after each matmul so the
                work can be spread one matmul per attention kc-slot."""
                w_t = wq_t[jc] if which == "q" else wk_t[jc]
                dst = qd[jc] if which == "q" else kd[jc]
                bias = bq_sb if which == "q" else bk_sb
                pps = ps.tile([P, 512], F32, tag="proj", bufs=1, name="pps")
                for dc in range(DC):
                    nc.tensor.matmul(
                        pps,
                        w_t[:, dc, :],
                        ht_t[:, dc, :],
                        start=(dc == 0),
                        stop=(dc == DC - 1),
                    )
                    if step and dc < DC - 1:
                        yield
                nc.vector.tensor_scalar_add(
                    dst[:, tb * 512 : (tb + 1) * 512], pps, bias[:, jc : jc + 1]
                )
                if step:
                    yield

            # ===== lead-in: V proj (all heads) + Q/K proj for jc=0 =====
            for tb in range(TB):
                ht_t = ht_first if tb == 0 else load_ht(tb)
                # K/Q first: they gate the first attention S-matmuls
                for gen in (
                    emit_qk_proj(0, tb, ht_t, "k"),
                    emit_qk_proj(0, tb, ht_t, "q"),
                ):
                    for _ in gen:
                        pass
                # V proj: out [t(part), j] ; lhsT = ht chunk [d, t128]
                for tcl in range(4):
                    tg = tb * 4 + tcl
                    vps = ps.tile([P, DJ], F32, tag="proj", bufs=1, name="vps")
                    for dc in range(DC):
                        nc.tensor.matmul(
                            vps,
                            ht_t[:, dc, tcl * P : (tcl + 1) * P],
                            wv_sb[:, dc, :],
                            start=(dc == 0),
                            stop=(dc == DC - 1),
                        )
                    nc.vector.tensor_copy(
                        v_all[:, tg, :, 0:HD],
                        vps.rearrange("p (h j) -> p h j", j=HD),
                    )

            # ===== deferred proj for jc 1..3, one matmul per next() =====
            def proj_gen():
                for jc in (1, 2, 3):
                    for tb in range(TB):
                        ht_t = load_ht(tb)
                        if tb == 0 and jc < 3:
                            load_wjc(jc + 1)
                        yield from emit_qk_proj(jc, tb, ht_t, "k", step=True)
                        yield from emit_qk_proj(jc, tb, ht_t, "q", step=True)

            pgen = proj_gen()
            dummy = {"ps": None, "n": 0}

            def emit_filler():
                """One PE matmul per kc-slot keeps the tensor engine dense
                (HAM stays warm): a deferred projection matmul while any
                remain (pairs 0-2), then cheap dummies (pair 3)."""
                try:
                    next(pgen)
                    return
                except StopIteration:
                    pass
                if dummy["ps"] is None:
                    dummy["ps"] = ps.tile(
                        [32, 384], F32, tag="proj", bufs=1, name="dummy_ps"
                    )
                nc.tensor.matmul(
                    dummy["ps"],
                    kd[3][0:HD, 0:32],
                    qd[3][0:HD, 0:384],
                    start=(dummy["n"] == 0),
                    stop=False,
                    skip_group_check=True,
                )
                dummy["n"] += 1

            # ===== attention: head pair (2p, 2p+1) per j-chunk p =====
            # S matmuls for the two heads go to PE row groups 0-63 / 64-127
            # (tile_position from base_partition) and run concurrently; one
            # ACTIVATE covers both heads' scores [128, 2x512].
            for p in range(JC):
                hA, hB = 2 * p, 2 * p + 1
                for q2 in range(S // 512):
                    q0 = q2 * 512
                    ctx_a = ps.tile([HD + 1, 512], F32, tag="ctxa", bufs=2)
                    ctx_b = ps.tile([HD + 1, 512], F32, tag="ctxb", bufs=1)
                    pend = None
                    for kc in range(KC):
                        s_ps = ps.tile([P, 1024], F32, tag="s", bufs=2, name="s_ps")
                        nc.tensor.matmul(
                            s_ps[:, 0:512],
                            kd[p][0:HD, kc * P : (kc + 1) * P],
                            qd[p][0:HD, q0 : q0 + 512],
                            start=True,
                            stop=True,
                        )
                        nc.tensor.matmul(
                            s_ps[:, 512:1024],
                            kd[p][HD:P, kc * P : (kc + 1) * P],
                            qd[p][HD:P, q0 : q0 + 512],
                            start=True,
                            stop=True,
                        )
                        e_sb = e_pool.tile([P, 1024], F32R, tag="e", name="e_sb")
                        nc.scalar.activation(
                            e_sb,
                            s_ps,
                            EXP,
                            bias=mask_sb[:, kc : kc + 1],
                            scale=1.0 / np.sqrt(HD),
                        )
                        # keep PE dense while ACT computes this exp
                        emit_filler()
                        # PV one step behind: next S-matmul isn't blocked
                        if pend is not None:
                            pkc, pe = pend
                            nc.tensor.matmul(
                                ctx_a,
                                v_all[:, pkc, hA, :],
                                pe[:, 0:512],
                                start=(pkc == 0),
                                stop=(pkc == KC - 1),
                            )
                            nc.tensor.matmul(
                                ctx_b,
                                v_all[:, pkc, hB, :],
                                pe[:, 512:1024],
                                start=(pkc == 0),
                                stop=(pkc == KC - 1),
                            )
                        pend = (kc, e_sb)
                    pkc, pe = pend
                    nc.tensor.matmul(
                        ctx_a,
                        v_all[:, pkc, hA, :],
                        pe[:, 0:512],
                        start=(pkc == 0),
                        stop=(pkc == KC - 1),
                    )
                    nc.tensor.matmul(
                        ctx_b,
                        v_all[:, pkc, hB, :],
                        pe[:, 512:1024],
                        start=(pkc == 0),
                        stop=(pkc == KC - 1),
                    )
                    out_a = o_pool.tile([HD + 1, 512], F32, tag="o", bufs=4)
                    nc.vector.tensor_copy(out_a, ctx_a)
                    nc.sync.dma_start(out_d.ap()[hA, :, q0 : q0 + 512], out_a)
                    out_b = o_pool.tile([HD + 1, 512], F32, tag="o", bufs=4)
                    nc.vector.tensor_copy(out_b, ctx_b)
                    nc.sync.dma_start(out_d.ap()[hB, :, q0 : q0 + 512], out_b)

            # drain any leftover deferred projection work
            for _ in pgen:
                pass
            # terminate + consume the dummy accumulator so it isn't dead
            if dummy["ps"] is not None:
                nc.tensor.matmul(
                    dummy["ps"],
                    kd[3][0:HD, 0:32],
                    qd[3][0:HD, 0:384],
                    start=False,
                    stop=True,
                    skip_group_check=True,
                )
                scr_sb = const_pool.tile([32, 384], F32)
                nc.vector.tensor_copy(scr_sb, dummy["ps"])
                nc.sync.dma_start(scr_d.ap(), scr_sb)

    nc.compile()
    return nc


def get_nc():
    if "nc" not in _CACHE:
        _CACHE["nc"] = build_nc()
    return _CACHE["nc"]


def make_in_maps(inputs):
    """Shard full inputs into per-core in_maps (host-side prep)."""
    hidden = np.asarray(inputs["hidden_states"], dtype=np.float32)
    mask = np.asarray(inputs["attention_mask"], dtype=np.float32)
    Wq = np.asarray(inputs["Wq"], dtype=np.float32)
    Wk = np.asarray(inputs["Wk"], dtype=np.float32)
    Wv = np.asarray(inputs["Wv"], dtype=np.float32)
    bq = np.asarray(inputs["bq"], dtype=np.float32)
    bk = np.asarray(inputs["bk"], dtype=np.float32)

    in_maps = []
    for c in range(N_CORES):
        b, hh = c // 2, c % 2
        js = slice(hh * DJ, (hh + 1) * DJ)
        in_maps.append(
            {
                "ht": np.ascontiguousarray(hidden[b].T),
                "wq": np.ascontiguousarray(Wq[:, js]),
                "wk": np.ascontiguousarray(Wk[:, js]),
                "wv": np.ascontiguousarray(Wv[:, js]),
                "bq": np.ascontiguousarray(bq[js]),
                "bk": np.ascontiguousarray(bk[js]),
                "mask": np.ascontiguousarray(mask[b, 0, 0]),
            }
        )
    return in_maps


def assemble_output(core_outs, bv):
    """core_outs: list of [NH, HD+1, S] arrays. Returns [B, S, D]."""
    bv = np.asarray(bv, dtype=np.float32)
    out = np.empty((B, S, D), dtype=np.float32)
    for c in range(N_CORES):
        b, hh = c // 2, c % 2
        arr = core_outs[c]  # [8, 65, 2048]
        ctx_u = arr[:, :HD, :]  # [8, 64, 2048]
        den = arr[:, HD, :]  # [8, 2048]
        bv_c = bv[hh * DJ : (hh + 1) * DJ].reshape(NH, HD)
        ctx = ctx_u / den[:, None, :] + bv_c[:, :, None]
        # [8 heads, 64 hd, 2048 t] -> [t, head, hd] -> [S, 512]
        out[b, :, hh * DJ : (hh + 1) * DJ] = (
            ctx.transpose(2, 0, 1).reshape(S, DJ)
        )
    return out


def kernel(**inputs):
    nc = get_nc()
    in_maps = make_in_maps(inputs)
    res = run_bass_kernel_spmd(
        nc,
        in_maps,
        core_ids=list(range(N_CORES)),
        trace=bool(int(os.environ.get("KERNEL_TRACE", "0"))),
    )
    if res.exec_time_ns is not None:
        print(f"HW exec time: {res.exec_time_ns} ns", file=sys.stderr)
        _CACHE["exec_time_ns"] = res.exec_time_ns
        _CACHE["results"] = res
    core_outs = [r["out"] for r in res.results]
    return assemble_output(core_outs, inputs["bv"])


# revision 38
# speedup vs baseline: 1.0182x; 1.0182x over previous
"""BERT self-attention Bass kernel for 8 Trainium2 NeuronCores.

Problem: B=4, S=2048, D=1024, H=16, HD=64 fp32.
Sharding: core c -> batch b=c//2, head-half hh=c%2 (heads hh*8..hh*8+8).

Per-core dataflow (everything fp32):
  - host pre-transposes hidden[b] -> ht [D=1024, T=2048] (d-major)
  - Q^T, K^T projections: out [j(part), t(free)] with lhsT = W chunks
  - V projection: out [t(part), j(free)] with lhsT = ht chunks; V stored
    per (t-chunk, head) as [128, 64] with a 65th column of ones so the
    PV matmul's 65th output row accumulates the softmax denominator.
  - scores computed transposed: S_t[k(part), q(free)] = K_d^T-chunk.T @ Q_d
    (contract over hd=64); exp on ScalarE directly from PSUM with
    scale=1/8 and per-partition bias = attention-mask slice.
  - PV: ctx^T[hd+1, q] += V_aug-chunk.T @ E_t-chunk, accumulated over the
    16 k-chunks in PSUM.
  - out[h] = [65, 2048] (unnormalized ctx^T plus denominator row).
Host: ctx = out[:64]/out[64] + bv (exact: sum of probs is 1), transpose,
interleave heads into [B, S, D].
"""

import os
import sys

import numpy as np

for p in ("/opt/trn_rl_repo", "/root/.axon_site", "/root/.axon_site/_ro/trn_rl_repo"):
    if os.path.isdir(p) and p not in sys.path:
        sys.path.append(p)

import concourse.bacc as bacc
import concourse.bass as bass
import concourse.mybir as mybir
import concourse.tile as tile
from concourse.bass_utils import run_bass_kernel_spmd

B, S, D, H = 4, 2048, 1024, 16
HD = D // H  # 64
N_CORES = 8
P = 128
DJ = 512  # per-core head columns (8 heads * 64)
NH = 8  # heads per core
DC = D // P  # 8 d-in chunks
JC = DJ // P  # 4 j chunks (2 heads each)
TB = 4  # t blocks of 512 in projection
KC = S // P  # 16 k chunks
QW = 1024  # q tile width in attention
QC = S // QW  # 2
F32 = mybir.dt.float32
F32R = mybir.dt.float32r  # reduced-precision PE input: 1 cycle/row vs 4

_CACHE = {}


def build_nc():
    """Build + compile the SPMD single-core program (same for all cores)."""
    nc = bacc.Bacc("TRN2", target_bir_lowering=False, debug=False)

    # f32r inputs: same fp32 bits from the host; PE rounds on read and the
    # BIR verifier accepts DMA-from-f32r-DRAM as a rounded producer.
    ht_d = nc.declare_dram_parameter("ht", [D, S], F32R, isOutput=False)
    wq_d = nc.declare_dram_parameter("wq", [D, DJ], F32R, isOutput=False)
    wk_d = nc.declare_dram_parameter("wk", [D, DJ], F32R, isOutput=False)
    wv_d = nc.declare_dram_parameter("wv", [D, DJ], F32R, isOutput=False)
    bq_d = nc.declare_dram_parameter("bq", [DJ], F32, isOutput=False)
    bk_d = nc.declare_dram_parameter("bk", [DJ], F32, isOutput=False)
    mask_d = nc.declare_dram_parameter("mask", [S], F32, isOutput=False)
    out_d = nc.declare_dram_parameter("out", [NH, HD + 1, S], F32, isOutput=True)

    scr_d = nc.dram_tensor("scr", [32, 384], F32)  # keeps dummy matmuls live

    EXP = mybir.ActivationFunctionType.Exp

    with tile.TileContext(nc) as tc:
        with (
            tc.tile_pool(name="const", bufs=1) as const_pool,
            tc.tile_pool(name="w", bufs=1) as w_pool,
            tc.tile_pool(name="qk", bufs=1) as qk_pool,
            tc.tile_pool(name="v", bufs=1) as v_pool,
            tc.tile_pool(name="ht", bufs=2) as ht_pool,
            tc.tile_pool(name="e", bufs=4) as e_pool,
            tc.tile_pool(name="o", bufs=2) as o_pool,
            tc.tile_pool(name="ps", bufs=1, space="PSUM") as ps,
        ):
            # ---- constants ----
            bq_sb = const_pool.tile([P, JC], F32)
            nc.sync.dma_start(bq_sb, bq_d.ap().rearrange("(jc p) -> p jc", p=P))
            bk_sb = const_pool.tile([P, JC], F32)
            nc.sync.dma_start(bk_sb, bk_d.ap().rearrange("(jc p) -> p jc", p=P))
            mask_sb = const_pool.tile([P, KC], F32)
            nc.sync.dma_start(mask_sb, mask_d.ap().rearrange("(kc p) -> p kc", p=P))

            # ---- weights: V full; Q/K per j-chunk (lazily, 2 slots) ----
            wq_r = wq_d.ap().rearrange("(dc p) j -> p dc j", p=P)
            wk_r = wk_d.ap().rearrange("(dc p) j -> p dc j", p=P)
            wq_t, wk_t = {}, {}

            def load_wjc(jc):
                tk = w_pool.tile([P, DC, P], F32R, tag="wkjc", bufs=2, name=f"wk{jc}")
                nc.sync.dma_start(tk, wk_r[:, :, jc * P : (jc + 1) * P])
                wk_t[jc] = tk
                tq = w_pool.tile([P, DC, P], F32R, tag="wqjc", bufs=2, name=f"wq{jc}")
                nc.sync.dma_start(tq, wq_r[:, :, jc * P : (jc + 1) * P])
                wq_t[jc] = tq

            ht_r = ht_d.ap().rearrange("(dc p) t -> p dc t", p=P)

            def load_ht(tb):
                t = ht_pool.tile([P, DC, 512], F32R, tag="ht", name="ht_t")
                ts_ = slice(tb * 512, (tb + 1) * 512)
                nc.sync.dma_start(t[:, 0:4, :], ht_r[:, 0:4, ts_])
                nc.sync.dma_start(t[:, 4:DC, :], ht_r[:, 4:DC, ts_])
                return t

            # DMA order matters: the first projection matmul needs wk0 + ht
            # tile 0, so issue those before the bulk weight loads (HWDGE is
            # FIFO per issuing engine).
            load_wjc(0)
            ht_first = load_ht(0)
            wv_sb = w_pool.tile([P, DC, DJ], F32R)
            wv_r = wv_d.ap().rearrange("(dc p) j -> p dc j", p=P)
            nc.sync.dma_start(wv_sb[:, 0:4, :], wv_r[:, 0:4, :])
            nc.sync.dma_start(wv_sb[:, 4:DC, :], wv_r[:, 4:DC, :])
            load_wjc(1)

            # ---- persistent activations ----
            # Q^T / K^T per j-chunk: [j in chunk (part), t (free)]
            qd = [qk_pool.tile([P, S], F32R, name=f"qd{jc}") for jc in range(JC)]
            kd = [qk_pool.tile([P, S], F32R, name=f"kd{jc}") for jc in range(JC)]
            # V + ones column: [t within chunk (part), kchunk, head, hd+1]
            v_all = v_pool.tile([P, KC, NH, HD + 1], F32R)
            ones_sb = const_pool.tile([P, KC * NH], F32)
            nc.vector.memset(ones_sb, 1.0)
            nc.vector.tensor_copy(
                v_all[:, :, :, HD],
                ones_sb.rearrange("p (a b) -> p a b", b=NH),
            )

            def emit_qk_proj(jc, tb, ht_t, which, step=False):
                """Project one [128j, 512t] tile of Q^T or K^T. When used as
                a generator (step=True) it yields after each matmul so the
                work can be spread one matmul per attention kc-slot."""
                w_t = wq_t[jc] if which == "q" else wk_t[jc]
                dst = qd[jc] if which == "q" else kd[jc]
                bias = bq_sb if which == "q" else bk_sb
                pps = ps.tile([P, 512], F32, tag="proj", bufs=2, name="pps")
                for dc in range(DC):
                    nc.tensor.matmul(
                        pps,
                        w_t[:, dc, :],
                        ht_t[:, dc, :],
                        start=(dc == 0),
                        stop=(dc == DC - 1),
                    )
                    if step and dc < DC - 1:
                        yield
                nc.vector.tensor_scalar_add(
                    dst[:, tb * 512 : (tb + 1) * 512], pps, bias[:, jc : jc + 1]
                )
                if step:
                    yield

            # ===== lead-in: V proj (all heads) + Q/K proj for jc=0 =====
            for tb in range(TB):
                ht_t = ht_first if tb == 0 else load_ht(tb)
                # K/Q first: they gate the first attention S-matmuls
                for gen in (
                    emit_qk_proj(0, tb, ht_t, "k"),
                    emit_qk_proj(0, tb, ht_t, "q"),
                ):
                    for _ in gen:
                        pass
                # V proj: out [t(part), j] ; lhsT = ht chunk [d, t128]
                for tcl in range(4):
                    tg = tb * 4 + tcl
                    vps = ps.tile([P, DJ], F32, tag="proj", bufs=2, name="vps")
                    for dc in range(DC):
                        nc.tensor.matmul(
                            vps,
                            ht_t[:, dc, tcl * P : (tcl + 1) * P],
                            wv_sb[:, dc, :],
                            start=(dc == 0),
                            stop=(dc == DC - 1),
                        )
                    nc.vector.tensor_copy(
                        v_all[:, tg, :, 0:HD],
                        vps.rearrange("p (h j) -> p h j", j=HD),
                    )

            # ===== deferred proj for jc 1..3, one matmul per next() =====
            def proj_gen():
                for jc in (1, 2, 3):
                    for tb in range(TB):
                        ht_t = load_ht(tb)
                        if tb == 0 and jc < 3:
                            load_wjc(jc + 1)
                        yield from emit_qk_proj(jc, tb, ht_t, "k", step=True)
                        yield from emit_qk_proj(jc, tb, ht_t, "q", step=True)

            pgen = proj_gen()
            dummy = {"ps": None, "n": 0}

            def emit_filler():
                """One PE matmul per kc-slot keeps the tensor engine dense
                (HAM stays warm): a deferred projection matmul while any
                remain (pairs 0-2), then cheap dummies (pair 3)."""
                try:
                    next(pgen)
                    return
                except StopIteration:
                    pass
                if dummy["ps"] is None:
                    dummy["ps"] = ps.tile(
                        [32, 384], F32, tag="proj", bufs=2, name="dummy_ps"
                    )
                nc.tensor.matmul(
                    dummy["ps"],
                    kd[3][0:HD, 0:32],
                    qd[3][0:HD, 0:384],
                    start=(dummy["n"] == 0),
                    stop=False,
                    skip_group_check=True,
                )
                dummy["n"] += 1

            # ===== attention: head pair (2p, 2p+1) per j-chunk p =====
            # S matmuls for the two heads go to PE row groups 0-63 / 64-127
            # (tile_position from base_partition) and run concurrently; one
            # ACTIVATE covers both heads' scores [128, 2x512].
            for p in range(JC):
                hA, hB = 2 * p, 2 * p + 1
                for q2 in range(S // 512):
                    q0 = q2 * 512
                    ctx_a = ps.tile([HD + 1, 512], F32, tag="ctxa", bufs=1)
                    ctx_b = ps.tile([HD + 1, 512], F32, tag="ctxb", bufs=1)
                    pend = None
                    for kc in range(KC):
                        s_ps = ps.tile([P, 1024], F32, tag="s", bufs=2, name="s_ps")
                        nc.tensor.matmul(
                            s_ps[:, 0:512],
                            kd[p][0:HD, kc * P : (kc + 1) * P],
                            qd[p][0:HD, q0 : q0 + 512],
                            start=True,
                            stop=True,
                        )
                        nc.tensor.matmul(
                            s_ps[:, 512:1024],
                            kd[p][HD:P, kc * P : (kc + 1) * P],
                            qd[p][HD:P, q0 : q0 + 512],
                            start=True,
                            stop=True,
                        )
                        e_sb = e_pool.tile([P, 1024], F32R, tag="e", name="e_sb")
                        nc.scalar.activation(
                            e_sb,
                            s_ps,
                            EXP,
                            bias=mask_sb[:, kc : kc + 1],
                            scale=1.0 / np.sqrt(HD),
                        )
                        # keep PE dense while ACT computes this exp
                        emit_filler()
                        # PV one step behind: next S-matmul isn't blocked
                        if pend is not None:
                            pkc, pe = pend
                            nc.tensor.matmul(
                                ctx_a,
                                v_all[:, pkc, hA, :],
                                pe[:, 0:512],
                                start=(pkc == 0),
                                stop=(pkc == KC - 1),
                            )
                            nc.tensor.matmul(
                                ctx_b,
                                v_all[:, pkc, hB, :],
                                pe[:, 512:1024],
                                start=(pkc == 0),
                                stop=(pkc == KC - 1),
                            )
                        pend = (kc, e_sb)
                    pkc, pe = pend
                    nc.tensor.matmul(
                        ctx_a,
                        v_all[:, pkc, hA, :],
                        pe[:, 0:512],
                        start=(pkc == 0),
                        stop=(pkc == KC - 1),
                    )
                    nc.tensor.matmul(
                        ctx_b,
                        v_all[:, pkc, hB, :],
                        pe[:, 512:1024],
                        start=(pkc == 0),
                        stop=(pkc == KC - 1),
                    )
                    out_a = o_pool.tile([HD + 1, 512], F32, tag="o", bufs=4)
                    nc.vector.tensor_copy(out_a, ctx_a)
                    nc.sync.dma_start(out_d.ap()[hA, :, q0 : q0 + 512], out_a)
                    out_b = o_pool.tile([HD + 1, 512], F32, tag="o", bufs=4)
                    nc.vector.tensor_copy(out_b, ctx_b)
                    nc.sync.dma_start(out_d.ap()[hB, :, q0 : q0 + 512], out_b)

            # drain any leftover deferred projection work
            for _ in pgen:
                pass
            # terminate + consume the dummy accumulator so it isn't dead
            if dummy["ps"] is not None:
                nc.tensor.matmul(
                    dummy["ps"],
                    kd[3][0:HD, 0:32],
                    qd[3][0:HD, 0:384],
                    start=False,
                    stop=True,
                    skip_group_check=True,
                )
                scr_sb = const_pool.tile([32, 384], F32)
                nc.vector.tensor_copy(scr_sb, dummy["ps"])
                nc.sync.dma_start(scr_d.ap(), scr_sb)

    nc.compile()
    return nc


def get_nc():
    if "nc" not in _CACHE:
        _CACHE["nc"] = build_nc()
    return _CACHE["nc"]


def make_in_maps(inputs):
    """Shard full inputs into per-core in_maps (host-side prep)."""
    hidden = np.asarray(inputs["hidden_states"], dtype=np.float32)
    mask = np.asarray(inputs["attention_mask"], dtype=np.float32)
    Wq = np.asarray(inputs["Wq"], dtype=np.float32)
    Wk = np.asarray(inputs["Wk"], dtype=np.float32)
    Wv = np.asarray(inputs["Wv"], dtype=np.float32)
    bq = np.asarray(inputs["bq"], dtype=np.float32)
    bk = np.asarray(inputs["bk"], dtype=np.float32)

    in_maps = []
    for c in range(N_CORES):
        b, hh = c // 2, c % 2
        js = slice(hh * DJ, (hh + 1) * DJ)
        in_maps.append(
            {
                "ht": np.ascontiguousarray(hidden[b].T),
                "wq": np.ascontiguousarray(Wq[:, js]),
                "wk": np.ascontiguousarray(Wk[:, js]),
                "wv": np.ascontiguousarray(Wv[:, js]),
                "bq": np.ascontiguousarray(bq[js]),
                "bk": np.ascontiguousarray(bk[js]),
                "mask": np.ascontiguousarray(mask[b, 0, 0]),
            }
        )
    return in_maps


def assemble_output(core_outs, bv):
    """core_outs: list of [NH, HD+1, S] arrays. Returns [B, S, D]."""
    bv = np.asarray(bv, dtype=np.float32)
    out = np.empty((B, S, D), dtype=np.float32)
    for c in range(N_CORES):
        b, hh = c // 2, c % 2
        arr = core_outs[c]  # [8, 65, 2048]
        ctx_u = arr[:, :HD, :]  # [8, 64, 2048]
        den = arr[:, HD, :]  # [8, 2048]
        bv_c = bv[hh * DJ : (hh + 1) * DJ].reshape(NH, HD)
        ctx = ctx_u / den[:, None, :] + bv_c[:, :, None]
        # [8 heads, 64 hd, 2048 t] -> [t, head, hd] -> [S, 512]
        out[b, :, hh * DJ : (hh + 1) * DJ] = (
            ctx.transpose(2, 0, 1).reshape(S, DJ)
        )
    return out


def kernel(**inputs):
    nc = get_nc()
    in_maps = make_in_maps(inputs)
    res = run_bass_kernel_spmd(
        nc,
        in_maps,
        core_ids=list(range(N_CORES)),
        trace=bool(int(os.environ.get("KERNEL_TRACE", "0"))),
    )
    if res.exec_time_ns is not None:
        print(f"HW exec time: {res.exec_time_ns} ns", file=sys.stderr)
        _CACHE["exec_time_ns"] = res.exec_time_ns
        _CACHE["results"] = res
    core_outs = [r["out"] for r in res.results]
    return assemble_output(core_outs, inputs["bv"])


# revision 41
# speedup vs baseline: 1.0243x; 1.0060x over previous
"""BERT self-attention Bass kernel for 8 Trainium2 NeuronCores.

Problem: B=4, S=2048, D=1024, H=16, HD=64 fp32.
Sharding: core c -> batch b=c//2, head-half hh=c%2 (heads hh*8..hh*8+8).

Per-core dataflow (everything fp32):
  - host pre-transposes hidden[b] -> ht [D=1024, T=2048] (d-major)
  - Q^T, K^T projections: out [j(part), t(free)] with lhsT = W chunks
  - V projection: out [t(part), j(free)] with lhsT = ht chunks; V stored
    per (t-chunk, head) as [128, 64] with a 65th column of ones so the
    PV matmul's 65th output row accumulates the softmax denominator.
  - scores computed transposed: S_t[k(part), q(free)] = K_d^T-chunk.T @ Q_d
    (contract over hd=64); exp on ScalarE directly from PSUM with
    scale=1/8 and per-partition bias = attention-mask slice.
  - PV: ctx^T[hd+1, q] += V_aug-chunk.T @ E_t-chunk, accumulated over the
    16 k-chunks in PSUM.
  - out[h] = [65, 2048] (unnormalized ctx^T plus denominator row).
Host: ctx = out[:64]/out[64] + bv (exact: sum of probs is 1), transpose,
interleave heads into [B, S, D].
"""

import os
import sys

import numpy as np

for p in ("/opt/trn_rl_repo", "/root/.axon_site", "/root/.axon_site/_ro/trn_rl_repo"):
    if os.path.isdir(p) and p not in sys.path:
        sys.path.append(p)

import concourse.bacc as bacc
import concourse.bass as bass
import concourse.mybir as mybir
import concourse.tile as tile
from concourse.bass_utils import run_bass_kernel_spmd

B, S, D, H = 4, 2048, 1024, 16
HD = D // H  # 64
N_CORES = 8
P = 128
DJ = 512  # per-core head columns (8 heads * 64)
NH = 8  # heads per core
DC = D // P  # 8 d-in chunks
JC = DJ // P  # 4 j chunks (2 heads each)
TB = 4  # t blocks of 512 in projection
KC = S // P  # 16 k chunks
QW = 1024  # q tile width in attention
QC = S // QW  # 2
F32 = mybir.dt.float32
F32R = mybir.dt.float32r  # reduced-precision PE input: 1 cycle/row vs 4

_CACHE = {}


def build_nc():
    """Build + compile the SPMD single-core program (same for all cores)."""
    nc = bacc.Bacc("TRN2", target_bir_lowering=False, debug=False)

    # f32r inputs: same fp32 bits from the host; PE rounds on read and the
    # BIR verifier accepts DMA-from-f32r-DRAM as a rounded producer.
    ht_d = nc.declare_dram_parameter("ht", [D, S], F32R, isOutput=False)
    wq_d = nc.declare_dram_parameter("wq", [D, DJ], F32R, isOutput=False)
    wk_d = nc.declare_dram_parameter("wk", [D, DJ], F32R, isOutput=False)
    wv_d = nc.declare_dram_parameter("wv", [D, DJ], F32R, isOutput=False)
    bq_d = nc.declare_dram_parameter("bq", [DJ], F32, isOutput=False)
    bk_d = nc.declare_dram_parameter("bk", [DJ], F32, isOutput=False)
    mask_d = nc.declare_dram_parameter("mask", [S], F32, isOutput=False)
    out_d = nc.declare_dram_parameter("out", [NH, HD + 1, S], F32, isOutput=True)

    scr_d = nc.dram_tensor("scr", [32, 128], F32)  # keeps dummy matmuls live

    EXP = mybir.ActivationFunctionType.Exp

    with tile.TileContext(nc) as tc:
        with (
            tc.tile_pool(name="const", bufs=1) as const_pool,
            tc.tile_pool(name="w", bufs=1) as w_pool,
            tc.tile_pool(name="qk", bufs=1) as qk_pool,
            tc.tile_pool(name="v", bufs=1) as v_pool,
            tc.tile_pool(name="ht", bufs=2) as ht_pool,
            tc.tile_pool(name="e", bufs=4) as e_pool,
            tc.tile_pool(name="o", bufs=2) as o_pool,
            tc.tile_pool(name="ps", bufs=1, space="PSUM") as ps,
        ):
            # ---- constants ----
            bq_sb = const_pool.tile([P, JC], F32)
            nc.sync.dma_start(bq_sb, bq_d.ap().rearrange("(jc p) -> p jc", p=P))
            bk_sb = const_pool.tile([P, JC], F32)
            nc.sync.dma_start(bk_sb, bk_d.ap().rearrange("(jc p) -> p jc", p=P))
            mask_sb = const_pool.tile([P, KC], F32)
            nc.sync.dma_start(mask_sb, mask_d.ap().rearrange("(kc p) -> p kc", p=P))

            # ---- weights: V full; Q/K per j-chunk (lazily, 2 slots) ----
            wq_r = wq_d.ap().rearrange("(dc p) j -> p dc j", p=P)
            wk_r = wk_d.ap().rearrange("(dc p) j -> p dc j", p=P)
            wq_t, wk_t = {}, {}

            def load_wjc(jc):
                tk = w_pool.tile([P, DC, P], F32R, tag="wkjc", bufs=2, name=f"wk{jc}")
                nc.sync.dma_start(tk, wk_r[:, :, jc * P : (jc + 1) * P])
                wk_t[jc] = tk
                tq = w_pool.tile([P, DC, P], F32R, tag="wqjc", bufs=2, name=f"wq{jc}")
                nc.sync.dma_start(tq, wq_r[:, :, jc * P : (jc + 1) * P])
                wq_t[jc] = tq

            ht_r = ht_d.ap().rearrange("(dc p) t -> p dc t", p=P)

            def load_ht(tb):
                t = ht_pool.tile([P, DC, 512], F32R, tag="ht", name="ht_t")
                ts_ = slice(tb * 512, (tb + 1) * 512)
                nc.sync.dma_start(t[:, 0:4, :], ht_r[:, 0:4, ts_])
                nc.sync.dma_start(t[:, 4:DC, :], ht_r[:, 4:DC, ts_])
                return t

            # DMA order matters: the first projection matmul needs wk0 + ht
            # tile 0, so issue those before the bulk weight loads (HWDGE is
            # FIFO per issuing engine).
            load_wjc(0)
            ht_first = load_ht(0)
            wv_sb = w_pool.tile([P, DC, DJ], F32R)
            wv_r = wv_d.ap().rearrange("(dc p) j -> p dc j", p=P)
            nc.sync.dma_start(wv_sb[:, 0:4, :], wv_r[:, 0:4, :])
            nc.sync.dma_start(wv_sb[:, 4:DC, :], wv_r[:, 4:DC, :])
            load_wjc(1)

            # ---- persistent activations ----
            # Q^T / K^T per j-chunk: [j in chunk (part), t (free)]
            qd = [qk_pool.tile([P, S], F32R, name=f"qd{jc}") for jc in range(JC)]
            kd = [qk_pool.tile([P, S], F32R, name=f"kd{jc}") for jc in range(JC)]
            # V + ones column: [t within chunk (part), kchunk, head, hd+1]
            v_all = v_pool.tile([P, KC, NH, HD + 1], F32R)
            ones_sb = const_pool.tile([P, KC * NH], F32)
            nc.vector.memset(ones_sb, 1.0)
            nc.vector.tensor_copy(
                v_all[:, :, :, HD],
                ones_sb.rearrange("p (a b) -> p a b", b=NH),
            )

            def emit_qk_proj(jc, tb, ht_t, which, step=False):
                """Project one [128j, 512t] tile of Q^T or K^T. When used as
                a generator (step=True) it yields after each matmul so the
                work can be spread one matmul per attention kc-slot."""
                w_t = wq_t[jc] if which == "q" else wk_t[jc]
                dst = qd[jc] if which == "q" else kd[jc]
                bias = bq_sb if which == "q" else bk_sb
                pps = ps.tile([P, 512], F32, tag="proj", bufs=2, name="pps")
                for dc in range(DC):
                    nc.tensor.matmul(
                        pps,
                        w_t[:, dc, :],
                        ht_t[:, dc, :],
                        start=(dc == 0),
                        stop=(dc == DC - 1),
                    )
                    if step and dc < DC - 1:
                        yield
                nc.vector.tensor_scalar_add(
                    dst[:, tb * 512 : (tb + 1) * 512], pps, bias[:, jc : jc + 1]
                )
                if step:
                    yield

            # ===== lead-in: V proj (all heads) + Q/K proj for jc=0 =====
            ht_next = ht_first
            for tb in range(TB):
                ht_t = ht_next
                if tb + 1 < TB:
                    ht_next = load_ht(tb + 1)  # prefetch behind current work
                # K/Q first: they gate the first attention S-matmuls
                for gen in (
                    emit_qk_proj(0, tb, ht_t, "k"),
                    emit_qk_proj(0, tb, ht_t, "q"),
                ):
                    for _ in gen:
                        pass
                # V proj: out [t(part), j] ; lhsT = ht chunk [d, t128]
                for tcl in range(4):
                    tg = tb * 4 + tcl
                    vps = ps.tile([P, DJ], F32, tag="proj", bufs=2, name="vps")
                    for dc in range(DC):
                        nc.tensor.matmul(
                            vps,
                            ht_t[:, dc, tcl * P : (tcl + 1) * P],
                            wv_sb[:, dc, :],
                            start=(dc == 0),
                            stop=(dc == DC - 1),
                        )
                    nc.vector.tensor_copy(
                        v_all[:, tg, :, 0:HD],
                        vps.rearrange("p (h j) -> p h j", j=HD),
                    )

            # ===== deferred proj for jc 1..3, one matmul per next() =====
            def proj_gen():
                # prefetch each group's ht tile one group ahead so the first
                # matmul of a group never waits on its 512KB DMA
                seq = [(jc, tb) for jc in (1, 2, 3) for tb in range(TB)]
                nxt = load_ht(seq[0][1])
                for i, (jc, tb) in enumerate(seq):
                    ht_t = nxt
                    if tb == 0 and jc < 3:
                        load_wjc(jc + 1)
                    if i + 1 < len(seq):
                        nxt = load_ht(seq[i + 1][1])
                    yield from emit_qk_proj(jc, tb, ht_t, "k", step=True)
                    yield from emit_qk_proj(jc, tb, ht_t, "q", step=True)

            pgen = proj_gen()
            dummy = {"ps": None, "n": 0}

            def emit_filler():
                """One PE matmul per kc-slot keeps the tensor engine dense
                (HAM stays warm): a deferred projection matmul while any
                remain (pairs 0-2), then cheap dummies (pair 3)."""
                try:
                    next(pgen)
                    return
                except StopIteration:
                    pass
                if dummy["ps"] is None:
                    dummy["ps"] = ps.tile(
                        [32, 128], F32, tag="proj", bufs=2, name="dummy_ps"
                    )
                nc.tensor.matmul(
                    dummy["ps"],
                    kd[3][0:HD, 0:32],
                    qd[3][0:HD, 0:128],
                    start=(dummy["n"] == 0),
                    stop=False,
                    skip_group_check=True,
                )
                dummy["n"] += 1

            # ===== attention: head pair (2p, 2p+1) per j-chunk p =====
            # S matmuls for the two heads go to PE row groups 0-63 / 64-127
            # (tile_position from base_partition) and run concurrently; one
            # ACTIVATE covers both heads' scores [128, 2x512].
            for p in range(JC):
                hA, hB = 2 * p, 2 * p + 1
                for q2 in range(S // 512):
                    q0 = q2 * 512
                    ctx_a = ps.tile([HD + 1, 512], F32, tag="ctxa", bufs=1)
                    ctx_b = ps.tile([HD + 1, 512], F32, tag="ctxb", bufs=1)
                    pend = None
                    for kc in range(KC):
                        s_ps = ps.tile([P, 1024], F32, tag="s", bufs=2, name="s_ps")
                        nc.tensor.matmul(
                            s_ps[:, 0:512],
                            kd[p][0:HD, kc * P : (kc + 1) * P],
                            qd[p][0:HD, q0 : q0 + 512],
                            start=True,
                            stop=True,
                        )
                        nc.tensor.matmul(
                            s_ps[:, 512:1024],
                            kd[p][HD:P, kc * P : (kc + 1) * P],
                            qd[p][HD:P, q0 : q0 + 512],
                            start=True,
                            stop=True,
                        )
                        e_sb = e_pool.tile([P, 1024], F32R, tag="e", name="e_sb")
                        nc.scalar.activation(
                            e_sb,
                            s_ps,
                            EXP,
                            bias=mask_sb[:, kc : kc + 1],
                            scale=1.0 / np.sqrt(HD),
                        )
                        # keep PE dense while ACT computes this exp
                        emit_filler()
                        # PV one step behind: next S-matmul isn't blocked
                        if pend is not None:
                            pkc, pe = pend
                            nc.tensor.matmul(
                                ctx_a,
                                v_all[:, pkc, hA, :],
                                pe[:, 0:512],
                                start=(pkc == 0),
                                stop=(pkc == KC - 1),
                            )
                            nc.tensor.matmul(
                                ctx_b,
                                v_all[:, pkc, hB, :],
                                pe[:, 512:1024],
                                start=(pkc == 0),
                                stop=(pkc == KC - 1),
                            )
                        pend = (kc, e_sb)
                    pkc, pe = pend
                    nc.tensor.matmul(
                        ctx_a,
                        v_all[:, pkc, hA, :],
                        pe[:, 0:512],
                        start=(pkc == 0),
                        stop=(pkc == KC - 1),
                    )
                    nc.tensor.matmul(
                        ctx_b,
                        v_all[:, pkc, hB, :],
                        pe[:, 512:1024],
                        start=(pkc == 0),
                        stop=(pkc == KC - 1),
                    )
                    out_a = o_pool.tile([HD + 1, 512], F32, tag="o", bufs=4)
                    nc.vector.tensor_copy(out_a, ctx_a)
                    nc.sync.dma_start(out_d.ap()[hA, :, q0 : q0 + 512], out_a)
                    out_b = o_pool.tile([HD + 1, 512], F32, tag="o", bufs=4)
                    nc.vector.tensor_copy(out_b, ctx_b)
                    nc.sync.dma_start(out_d.ap()[hB, :, q0 : q0 + 512], out_b)

            # drain any leftover deferred projection work
            for _ in pgen:
                pass
            # terminate + consume the dummy accumulator so it isn't dead
            if dummy["ps"] is not None:
                nc.tensor.matmul(
                    dummy["ps"],
                    kd[3][0:HD, 0:32],
                    qd[3][0:HD, 0:128],
                    start=False,
                    stop=True,
                    skip_group_check=True,
                )
                scr_sb = const_pool.tile([32, 128], F32)
                nc.vector.tensor_copy(scr_sb, dummy["ps"])
                nc.sync.dma_start(scr_d.ap(), scr_sb)

    nc.compile()
    return nc


def get_nc():
    if "nc" not in _CACHE:
        _CACHE["nc"] = build_nc()
    return _CACHE["nc"]


def make_in_maps(inputs):
    """Shard full inputs into per-core in_maps (host-side prep)."""
    hidden = np.asarray(inputs["hidden_states"], dtype=np.float32)
    mask = np.asarray(inputs["attention_mask"], dtype=np.float32)
    Wq = np.asarray(inputs["Wq"], dtype=np.float32)
    Wk = np.asarray(inputs["Wk"], dtype=np.float32)
    Wv = np.asarray(inputs["Wv"], dtype=np.float32)
    bq = np.asarray(inputs["bq"], dtype=np.float32)
    bk = np.asarray(inputs["bk"], dtype=np.float32)

    in_maps = []
    for c in range(N_CORES):
        b, hh = c // 2, c % 2
        js = slice(hh * DJ, (hh + 1) * DJ)
        in_maps.append(
            {
                "ht": np.ascontiguousarray(hidden[b].T),
                "wq": np.ascontiguousarray(Wq[:, js]),
                "wk": np.ascontiguousarray(Wk[:, js]),
                "wv": np.ascontiguousarray(Wv[:, js]),
                "bq": np.ascontiguousarray(bq[js]),
                "bk": np.ascontiguousarray(bk[js]),
                "mask": np.ascontiguousarray(mask[b, 0, 0]),
            }
        )
    return in_maps


def assemble_output(core_outs, bv):
    """core_outs: list of [NH, HD+1, S] arrays. Returns [B, S, D]."""
    bv = np.asarray(bv, dtype=np.float32)
    out = np.empty((B, S, D), dtype=np.float32)
    for c in range(N_CORES):
        b, hh = c // 2, c % 2
        arr = core_outs[c]  # [8, 65, 2048]
        ctx_u = arr[:, :HD, :]  # [8, 64, 2048]
        den = arr[:, HD, :]  # [8, 2048]
        bv_c = bv[hh * DJ : (hh + 1) * DJ].reshape(NH, HD)
        ctx = ctx_u / den[:, None, :] + bv_c[:, :, None]
        # [8 heads, 64 hd, 2048 t] -> [t, head, hd] -> [S, 512]
        out[b, :, hh * DJ : (hh + 1) * DJ] = (
            ctx.transpose(2, 0, 1).reshape(S, DJ)
        )
    return out


def kernel(**inputs):
    nc = get_nc()
    in_maps = make_in_maps(inputs)
    res = run_bass_kernel_spmd(
        nc,
        in_maps,
        core_ids=list(range(N_CORES)),
        trace=bool(int(os.environ.get("KERNEL_TRACE", "0"))),
    )
    if res.exec_time_ns is not None:
        print(f"HW exec time: {res.exec_time_ns} ns", file=sys.stderr)
        _CACHE["exec_time_ns"] = res.exec_time_ns
        _CACHE["results"] = res
    core_outs = [r["out"] for r in res.results]
    return assemble_output(core_outs, inputs["bv"])


# revision 43
# speedup vs baseline: 1.0273x; 1.0030x over previous
"""BERT self-attention Bass kernel for 8 Trainium2 NeuronCores.

Problem: B=4, S=2048, D=1024, H=16, HD=64 fp32.
Sharding: core c -> batch b=c//2, head-half hh=c%2 (heads hh*8..hh*8+8).

Per-core dataflow (everything fp32):
  - host pre-transposes hidden[b] -> ht [D=1024, T=2048] (d-major)
  - Q^T, K^T projections: out [j(part), t(free)] with lhsT = W chunks
  - V projection: out [t(part), j(free)] with lhsT = ht chunks; V stored
    per (t-chunk, head) as [128, 64] with a 65th column of ones so the
    PV matmul's 65th output row accumulates the softmax denominator.
  - scores computed transposed: S_t[k(part), q(free)] = K_d^T-chunk.T @ Q_d
    (contract over hd=64); exp on ScalarE directly from PSUM with
    scale=1/8 and per-partition bias = attention-mask slice.
  - PV: ctx^T[hd+1, q] += V_aug-chunk.T @ E_t-chunk, accumulated over the
    16 k-chunks in PSUM.
  - out[h] = [65, 2048] (unnormalized ctx^T plus denominator row).
Host: ctx = out[:64]/out[64] + bv (exact: sum of probs is 1), transpose,
interleave heads into [B, S, D].
"""

import os
import sys

import numpy as np

for p in ("/opt/trn_rl_repo", "/root/.axon_site", "/root/.axon_site/_ro/trn_rl_repo"):
    if os.path.isdir(p) and p not in sys.path:
        sys.path.append(p)

import concourse.bacc as bacc
import concourse.bass as bass
import concourse.mybir as mybir
import concourse.tile as tile
from concourse.bass_utils import run_bass_kernel_spmd

B, S, D, H = 4, 2048, 1024, 16
HD = D // H  # 64
N_CORES = 8
P = 128
DJ = 512  # per-core head columns (8 heads * 64)
NH = 8  # heads per core
DC = D // P  # 8 d-in chunks
JC = DJ // P  # 4 j chunks (2 heads each)
TB = 4  # t blocks of 512 in projection
KC = S // P  # 16 k chunks
QW = 1024  # q tile width in attention
QC = S // QW  # 2
F32 = mybir.dt.float32
F32R = mybir.dt.float32r  # reduced-precision PE input: 1 cycle/row vs 4

_CACHE = {}


def build_nc():
    """Build + compile the SPMD single-core program (same for all cores)."""
    nc = bacc.Bacc("TRN2", target_bir_lowering=False, debug=False)

    # f32r inputs: same fp32 bits from the host; PE rounds on read and the
    # BIR verifier accepts DMA-from-f32r-DRAM as a rounded producer.
    ht_d = nc.declare_dram_parameter("ht", [D, S], F32R, isOutput=False)
    wq_d = nc.declare_dram_parameter("wq", [D, DJ], F32R, isOutput=False)
    wk_d = nc.declare_dram_parameter("wk", [D, DJ], F32R, isOutput=False)
    wv_d = nc.declare_dram_parameter("wv", [D, DJ], F32R, isOutput=False)
    bq_d = nc.declare_dram_parameter("bq", [DJ], F32, isOutput=False)
    bk_d = nc.declare_dram_parameter("bk", [DJ], F32, isOutput=False)
    mask_d = nc.declare_dram_parameter("mask", [S], F32, isOutput=False)
    out_d = nc.declare_dram_parameter("out", [NH, HD + 1, S], F32, isOutput=True)

    scr_d = nc.dram_tensor("scr", [32, 128], F32)  # keeps dummy matmuls live

    EXP = mybir.ActivationFunctionType.Exp

    with tile.TileContext(nc) as tc:
        with (
            tc.tile_pool(name="const", bufs=1) as const_pool,
            tc.tile_pool(name="w", bufs=1) as w_pool,
            tc.tile_pool(name="qk", bufs=1) as qk_pool,
            tc.tile_pool(name="v", bufs=1) as v_pool,
            tc.tile_pool(name="ht", bufs=2) as ht_pool,
            tc.tile_pool(name="e", bufs=4) as e_pool,
            tc.tile_pool(name="o", bufs=2) as o_pool,
            tc.tile_pool(name="ps", bufs=1, space="PSUM") as ps,
        ):
            # ---- constants ----
            bq_sb = const_pool.tile([P, JC], F32)
            nc.sync.dma_start(bq_sb, bq_d.ap().rearrange("(jc p) -> p jc", p=P))
            bk_sb = const_pool.tile([P, JC], F32)
            nc.sync.dma_start(bk_sb, bk_d.ap().rearrange("(jc p) -> p jc", p=P))
            mask_sb = const_pool.tile([P, KC], F32)
            nc.sync.dma_start(mask_sb, mask_d.ap().rearrange("(kc p) -> p kc", p=P))

            # ---- weights: V full; Q/K per j-chunk (lazily, 2 slots) ----
            wq_r = wq_d.ap().rearrange("(dc p) j -> p dc j", p=P)
            wk_r = wk_d.ap().rearrange("(dc p) j -> p dc j", p=P)
            wq_t, wk_t = {}, {}

            def load_wjc(jc):
                tk = w_pool.tile([P, DC, P], F32R, tag="wkjc", bufs=2, name=f"wk{jc}")
                js = slice(jc * P, (jc + 1) * P)
                if jc == 0:
                    # split out dc0 so the very first matmul starts sooner
                    nc.sync.dma_start(tk[:, 0:1, :], wk_r[:, 0:1, js])
                    nc.sync.dma_start(tk[:, 1:DC, :], wk_r[:, 1:DC, js])
                else:
                    nc.sync.dma_start(tk, wk_r[:, :, js])
                wk_t[jc] = tk
                tq = w_pool.tile([P, DC, P], F32R, tag="wqjc", bufs=2, name=f"wq{jc}")
                nc.sync.dma_start(tq, wq_r[:, :, js])
                wq_t[jc] = tq

            ht_r = ht_d.ap().rearrange("(dc p) t -> p dc t", p=P)

            def load_ht(tb):
                t = ht_pool.tile([P, DC, 512], F32R, tag="ht", name="ht_t")
                ts_ = slice(tb * 512, (tb + 1) * 512)
                nc.sync.dma_start(t[:, 0:4, :], ht_r[:, 0:4, ts_])
                nc.sync.dma_start(t[:, 4:DC, :], ht_r[:, 4:DC, ts_])
                return t

            # DMA order matters: the first projection matmul needs wk0 + ht
            # tile 0, so issue those before the bulk weight loads (HWDGE is
            # FIFO per issuing engine).
            load_wjc(0)
            ht_first = ht_pool.tile([P, DC, 512], F32R, tag="ht", name="ht_t")
            nc.sync.dma_start(ht_first[:, 0:1, :], ht_r[:, 0:1, 0:512])
            nc.sync.dma_start(ht_first[:, 1:4, :], ht_r[:, 1:4, 0:512])
            nc.sync.dma_start(ht_first[:, 4:DC, :], ht_r[:, 4:DC, 0:512])
            wv_sb = w_pool.tile([P, DC, DJ], F32R)
            wv_r = wv_d.ap().rearrange("(dc p) j -> p dc j", p=P)
            nc.sync.dma_start(wv_sb[:, 0:4, :], wv_r[:, 0:4, :])
            nc.sync.dma_start(wv_sb[:, 4:DC, :], wv_r[:, 4:DC, :])
            load_wjc(1)

            # ---- persistent activations ----
            # Q^T / K^T per j-chunk: [j in chunk (part), t (free)]
            qd = [qk_pool.tile([P, S], F32R, name=f"qd{jc}") for jc in range(JC)]
            kd = [qk_pool.tile([P, S], F32R, name=f"kd{jc}") for jc in range(JC)]
            # V + ones column: [t within chunk (part), kchunk, head, hd+1]
            v_all = v_pool.tile([P, KC, NH, HD + 1], F32R)
            ones_sb = const_pool.tile([P, KC * NH], F32)
            nc.vector.memset(ones_sb, 1.0)
            nc.vector.tensor_copy(
                v_all[:, :, :, HD],
                ones_sb.rearrange("p (a b) -> p a b", b=NH),
            )

            def emit_qk_proj(jc, tb, ht_t, which, step=False):
                """Project one [128j, 512t] tile of Q^T or K^T. When used as
                a generator (step=True) it yields after each matmul so the
                work can be spread one matmul per attention kc-slot."""
                w_t = wq_t[jc] if which == "q" else wk_t[jc]
                dst = qd[jc] if which == "q" else kd[jc]
                bias = bq_sb if which == "q" else bk_sb
                pps = ps.tile([P, 512], F32, tag="proj", bufs=2, name="pps")
                for dc in range(DC):
                    nc.tensor.matmul(
                        pps,
                        w_t[:, dc, :],
                        ht_t[:, dc, :],
                        start=(dc == 0),
                        stop=(dc == DC - 1),
                    )
                    if step and dc < DC - 1:
                        yield
                nc.vector.tensor_scalar_add(
                    dst[:, tb * 512 : (tb + 1) * 512], pps, bias[:, jc : jc + 1]
                )
                if step:
                    yield

            # ===== lead-in: V proj (all heads) + Q/K proj for jc=0 =====
            ht_next = ht_first
            for tb in range(TB):
                ht_t = ht_next
                if tb + 1 < TB:
                    ht_next = load_ht(tb + 1)  # prefetch behind current work
                # K/Q first: they gate the first attention S-matmuls
                for gen in (
                    emit_qk_proj(0, tb, ht_t, "k"),
                    emit_qk_proj(0, tb, ht_t, "q"),
                ):
                    for _ in gen:
                        pass
                # V proj: out [t(part), j] ; lhsT = ht chunk [d, t128]
                for tcl in range(4):
                    tg = tb * 4 + tcl
                    vps = ps.tile([P, DJ], F32, tag="proj", bufs=2, name="vps")
                    for dc in range(DC):
                        nc.tensor.matmul(
                            vps,
                            ht_t[:, dc, tcl * P : (tcl + 1) * P],
                            wv_sb[:, dc, :],
                            start=(dc == 0),
                            stop=(dc == DC - 1),
                        )
                    nc.vector.tensor_copy(
                        v_all[:, tg, :, 0:HD],
                        vps.rearrange("p (h j) -> p h j", j=HD),
                    )

            # ===== deferred proj for jc 1..3, one matmul per next() =====
            def proj_gen():
                # prefetch each group's ht tile one group ahead so the first
                # matmul of a group never waits on its 512KB DMA
                seq = [(jc, tb) for jc in (1, 2, 3) for tb in range(TB)]
                nxt = load_ht(seq[0][1])
                for i, (jc, tb) in enumerate(seq):
                    ht_t = nxt
                    if tb == 0 and jc < 3:
                        load_wjc(jc + 1)
                    if i + 1 < len(seq):
                        nxt = load_ht(seq[i + 1][1])
                    yield from emit_qk_proj(jc, tb, ht_t, "k", step=True)
                    yield from emit_qk_proj(jc, tb, ht_t, "q", step=True)

            pgen = proj_gen()
            dummy = {"ps": None, "n": 0}

            def emit_filler():
                """One PE matmul per kc-slot keeps the tensor engine dense
                (HAM stays warm): a deferred projection matmul while any
                remain (pairs 0-2), then cheap dummies (pair 3)."""
                try:
                    next(pgen)
                    return
                except StopIteration:
                    pass
                if dummy["ps"] is None:
                    dummy["ps"] = ps.tile(
                        [32, 128], F32, tag="proj", bufs=2, name="dummy_ps"
                    )
                nc.tensor.matmul(
                    dummy["ps"],
                    kd[3][0:HD, 0:32],
                    qd[3][0:HD, 0:128],
                    start=(dummy["n"] == 0),
                    stop=False,
                    skip_group_check=True,
                )
                dummy["n"] += 1

            # ===== attention: head pair (2p, 2p+1) per j-chunk p =====
            # S matmuls for the two heads go to PE row groups 0-63 / 64-127
            # (tile_position from base_partition) and run concurrently; one
            # ACTIVATE covers both heads' scores [128, 2x512].
            for p in range(JC):
                hA, hB = 2 * p, 2 * p + 1
                for q2 in range(S // 512):
                    q0 = q2 * 512
                    ctx_a = ps.tile([HD + 1, 512], F32, tag="ctxa", bufs=1)
                    ctx_b = ps.tile([HD + 1, 512], F32, tag="ctxb", bufs=1)
                    pend = None
                    for kc in range(KC):
                        s_ps = ps.tile([P, 1024], F32, tag="s", bufs=2, name="s_ps")
                        nc.tensor.matmul(
                            s_ps[:, 0:512],
                            kd[p][0:HD, kc * P : (kc + 1) * P],
                            qd[p][0:HD, q0 : q0 + 512],
                            start=True,
                            stop=True,
                        )
                        nc.tensor.matmul(
                            s_ps[:, 512:1024],
                            kd[p][HD:P, kc * P : (kc + 1) * P],
                            qd[p][HD:P, q0 : q0 + 512],
                            start=True,
                            stop=True,
                        )
                        e_sb = e_pool.tile([P, 1024], F32R, tag="e", name="e_sb")
                        nc.scalar.activation(
                            e_sb,
                            s_ps,
                            EXP,
                            bias=mask_sb[:, kc : kc + 1],
                            scale=1.0 / np.sqrt(HD),
                        )
                        # keep PE dense while ACT computes this exp
                        emit_filler()
                        # PV one step behind: next S-matmul isn't blocked
                        if pend is not None:
                            pkc, pe = pend
                            nc.tensor.matmul(
                                ctx_a,
                                v_all[:, pkc, hA, :],
                                pe[:, 0:512],
                                start=(pkc == 0),
                                stop=(pkc == KC - 1),
                            )
                            nc.tensor.matmul(
                                ctx_b,
                                v_all[:, pkc, hB, :],
                                pe[:, 512:1024],
                                start=(pkc == 0),
                                stop=(pkc == KC - 1),
                            )
                        pend = (kc, e_sb)
                    pkc, pe = pend
                    nc.tensor.matmul(
                        ctx_a,
                        v_all[:, pkc, hA, :],
                        pe[:, 0:512],
                        start=(pkc == 0),
                        stop=(pkc == KC - 1),
                    )
                    nc.tensor.matmul(
                        ctx_b,
                        v_all[:, pkc, hB, :],
                        pe[:, 512:1024],
                        start=(pkc == 0),
                        stop=(pkc == KC - 1),
                    )
                    out_a = o_pool.tile([HD + 1, 512], F32, tag="o", bufs=4)
                    nc.vector.tensor_copy(out_a, ctx_a)
                    nc.sync.dma_start(out_d.ap()[hA, :, q0 : q0 + 512], out_a)
                    out_b = o_pool.tile([HD + 1, 512], F32, tag="o", bufs=4)
                    nc.vector.tensor_copy(out_b, ctx_b)
                    nc.sync.dma_start(out_d.ap()[hB, :, q0 : q0 + 512], out_b)

            # drain any leftover deferred projection work
            for _ in pgen:
                pass
            # terminate + consume the dummy accumulator so it isn't dead
            if dummy["ps"] is not None:
                nc.tensor.matmul(
                    dummy["ps"],
                    kd[3][0:HD, 0:32],
                    qd[3][0:HD, 0:128],
                    start=False,
                    stop=True,
                    skip_group_check=True,
                )
                scr_sb = const_pool.tile([32, 128], F32)
                nc.vector.tensor_copy(scr_sb, dummy["ps"])
                nc.sync.dma_start(scr_d.ap(), scr_sb)

    nc.compile()
    return nc


def get_nc():
    if "nc" not in _CACHE:
        _CACHE["nc"] = build_nc()
    return _CACHE["nc"]


def make_in_maps(inputs):
    """Shard full inputs into per-core in_maps (host-side prep)."""
    hidden = np.asarray(inputs["hidden_states"], dtype=np.float32)
    mask = np.asarray(inputs["attention_mask"], dtype=np.float32)
    Wq = np.asarray(inputs["Wq"], dtype=np.float32)
    Wk = np.asarray(inputs["Wk"], dtype=np.float32)
    Wv = np.asarray(inputs["Wv"], dtype=np.float32)
    bq = np.asarray(inputs["bq"], dtype=np.float32)
    bk = np.asarray(inputs["bk"], dtype=np.float32)

    in_maps = []
    for c in range(N_CORES):
        b, hh = c // 2, c % 2
        js = slice(hh * DJ, (hh + 1) * DJ)
        in_maps.append(
            {
                "ht": np.ascontiguousarray(hidden[b].T),
                "wq": np.ascontiguousarray(Wq[:, js]),
                "wk": np.ascontiguousarray(Wk[:, js]),
                "wv": np.ascontiguousarray(Wv[:, js]),
                "bq": np.ascontiguousarray(bq[js]),
                "bk": np.ascontiguousarray(bk[js]),
                "mask": np.ascontiguousarray(mask[b, 0, 0]),
            }
        )
    return in_maps


def assemble_output(core_outs, bv):
    """core_outs: list of [NH, HD+1, S] arrays. Returns [B, S, D]."""
    bv = np.asarray(bv, dtype=np.float32)
    out = np.empty((B, S, D), dtype=np.float32)
    for c in range(N_CORES):
        b, hh = c // 2, c % 2
        arr = core_outs[c]  # [8, 65, 2048]
        ctx_u = arr[:, :HD, :]  # [8, 64, 2048]
        den = arr[:, HD, :]  # [8, 2048]
        bv_c = bv[hh * DJ : (hh + 1) * DJ].reshape(NH, HD)
        ctx = ctx_u / den[:, None, :] + bv_c[:, :, None]
        # [8 heads, 64 hd, 2048 t] -> [t, head, hd] -> [S, 512]
        out[b, :, hh * DJ : (hh + 1) * DJ] = (
            ctx.transpose(2, 0, 1).reshape(S, DJ)
        )
    return out


def kernel(**inputs):
    nc = get_nc()
    in_maps = make_in_maps(inputs)
    res = run_bass_kernel_spmd(
        nc,
        in_maps,
        core_ids=list(range(N_CORES)),
        trace=bool(int(os.environ.get("KERNEL_TRACE", "0"))),
    )
    if res.exec_time_ns is not None:
        print(f"HW exec time: {res.exec_time_ns} ns", file=sys.stderr)
        _CACHE["exec_time_ns"] = res.exec_time_ns
        _CACHE["results"] = res
    core_outs = [r["out"] for r in res.results]
    return assemble_output(core_outs, inputs["bv"])
